# revision 1
# baseline (speedup 1.0000x reference)
"""Adaptive 7x7 Gaussian filter (softmax tap weights) on 8 TRN2 NeuronCores.

Math: per pixel, tap weight at offset (dr,dc) is softmax over 49 taps of
-((dr^2+dc^2)/2)*sigma^2.  With t = exp(-sigma^2/2) the unnormalized
weight is t^(dr^2+dc^2); the normalizer factorizes: Z = g^2 with
g = 1 + 2t + 2t^4 + 2t^9.  Grouping taps by squared distance
d in {0,1,2,4,5,8,9,10,13,18} with binary-stencil sums S_d and
u=t, v=t^4, w=t^9:

  out = (S0 + u*(S1 + u*S2)
             + v*(S4 + u*S5 + v*S8)
             + w*(S9 + u*S10 + v*S13 + w*S18)) * exp(-2*ln(g))

Engine split (vs the all-DVE baseline at 136us):
- TensorE (idle in baseline) builds S5/S10/S13 and the partial sums
  A1 = S1 + u*S2, A2 = S4 + u*S5 + v*S8, A3 = S9 + u*S10 + v*S13 + w*S18
  and outer = x + u*A1 + v*A2 + w*A3 as identity-weight matmuls over
  shifted rhs views, accumulating in PSUM (~223ns per 512-col matmul).
- DVE keeps V1..V3, cheap C maps, and per-pixel weight multiplies.
- ScalarE does casts, exp/ln, and PSUM->SBUF fp16 copies of S maps,
  with the exps interleaved between copy batches so neither consumer
  stalls.
- GpSimd adds half of g = u+v+w; DVE adds the other half.
- x load + cast split into edge/interior pieces so the reflect/halo
  chain starts ~10us earlier; sigma/aux loads go on separate DMA
  queues; the output is stored per column-quarter as it finishes.

Layout per core: 2 images x 3 channels = 6 planes of 256x256; each
plane split into 16 bands of 16 rows -> 96 partitions.  A partition
holds its band padded to [22 rows x 262 cols] (3 halo rows, reflect
cols), so BOTH stencil directions are free-dim AP offsets.  fp16
compute (DVE 2x dual-pump), f32 PSUM accumulation.
"""

import sys

sys.path.insert(0, "/opt/trn_rl_repo")

import numpy as np

import concourse.bacc as bacc
import concourse.bass as bass
import concourse.mybir as mybir
import concourse.tile as tile
from concourse.bass_utils import run_bass_kernel_spmd

B, CH, H, W = 16, 3, 256, 256
N_CORES = 8
B_PER_CORE = B // N_CORES          # 2 images per core
PLANES = B_PER_CORE * CH           # 6 planes per core
BANDS = 16                         # bands per plane
BR = H // BANDS                    # 16 rows per band
P = PLANES * BANDS                 # 96 partitions
PADR = BR + 6                      # 22 padded rows
PADC = W + 6                       # 262 padded cols
FMAP = BR * W                      # 4096 elems per partition per map
NQ = 4                             # column-quarters for PSUM staging
QF = FMAP // NQ                    # 1024 elems per quarter
QR = BR // NQ                      # 4 band-rows per quarter

F16 = mybir.dt.float16
F32 = mybir.dt.float32
AF = mybir.ActivationFunctionType


def build_nc():
    nc = bacc.Bacc(None, target_bir_lowering=False)
    x_d = nc.declare_dram_parameter("x", [B_PER_CORE, CH, H, W], F32, isOutput=False)
    s_d = nc.declare_dram_parameter("sigma", [B_PER_CORE, CH, H, W], F32, isOutput=False)
    o_d = nc.declare_dram_parameter("out", [B_PER_CORE, CH, H, W], F32, isOutput=True)
    # aux matrices, stacked [96, 5*96] = [top-shift, top-self, bot-shift,
    # bot-self, identity]
    sh_d = nc.declare_dram_parameter("hshift", [P, 5 * P], F16, isOutput=False)

    xv = x_d[:].rearrange("b ch (bd r) c -> (b ch bd) r c", r=BR)
    sv = s_d[:].rearrange("b ch (bd r) c -> (b ch bd) r c", r=BR)
    ov = o_d[:].rearrange("b ch (bd r) c -> (b ch bd) r c", r=BR)

    with tile.TileContext(nc) as tc:
        with (
            tc.tile_pool(name="io", bufs=2) as io,
            tc.tile_pool(name="xp", bufs=1) as xpp,
            tc.tile_pool(name="maps", bufs=4) as maps,
            tc.tile_pool(name="vp", bufs=3) as vp,
            tc.tile_pool(name="work", bufs=4) as work,
            tc.tile_pool(name="tq", bufs=1) as tqp,
            tc.tile_pool(name="pp", bufs=1) as ppp,
            tc.tile_pool(name="ssb", bufs=4) as ssbp,
            tc.tile_pool(name="psum", bufs=4, space="PSUM") as psp,
        ):
            xpad = xpp.tile([P, PADR * PADC], F16, tag="xpad", name="xpad")
            xpv = xpad[:].rearrange("p (r c) -> p r c", r=PADR)
            shmat = xpp.tile([P, 5 * P], F16, tag="shm", name="shmat")
            ident = shmat[:, 4 * P : 5 * P]

            def iof(name):
                return io.tile([P, FMAP], F32, tag="iof32", name=name)

            def r3(t):
                return t[:].rearrange("p (r c) -> p r c", r=BR)

            # ---- loads: x in 3 pieces on sync queue; sigma/aux on their
            # own queues so they overlap ----
            xf32 = iof("xf32")
            xf3 = r3(xf32)
            nc.sync.dma_start(out=xf3[:, 0:8], in_=xv[:, 0:8])
            nc.scalar.dma_start(out=xf3[:, 8:16], in_=xv[:, 8:16])
            sf32 = iof("sf32")
            nc.gpsimd.dma_start(out=shmat[:], in_=sh_d[:])
            nc.gpsimd.dma_start(out=r3(sf32), in_=sv)

            # ---- edge casts + edge col pads (ACT) ----
            nc.scalar.copy(out=xpv[:, 3:7, 3 : 3 + W], in_=xf3[:, 0:4])
            nc.scalar.copy(out=xpv[:, 15:19, 3 : 3 + W], in_=xf3[:, 12:16])
            for rr in ((3, 7), (15, 19)):
                nc.scalar.copy(out=xpv[:, rr[0] : rr[1], 0:3], in_=xpv[:, rr[0] : rr[1], 6:3:-1])
                nc.scalar.copy(
                    out=xpv[:, rr[0] : rr[1], 259:262],
                    in_=xpv[:, rr[0] : rr[1], 257:254:-1],
                )
            # self-reflect halo rows on GpSimd (correct for plane-edge bands;
            # interior bands overwritten by the halo exchange below)
            nc.gpsimd.tensor_copy(out=xpv[:, 0:3, :], in_=xpv[:, 6:3:-1, :])
            nc.gpsimd.tensor_copy(out=xpv[:, 19:22, :], in_=xpv[:, 17:14:-1, :])

            # ---- interior cast + pads (ACT), overlaps halo matmuls ----
            nc.scalar.copy(out=xpv[:, 7:11, 3 : 3 + W], in_=xf3[:, 4:8])
            nc.scalar.copy(out=xpv[:, 11:15, 3 : 3 + W], in_=xf3[:, 8:12])
            nc.scalar.copy(out=xpv[:, 7:15, 0:3], in_=xpv[:, 7:15, 6:3:-1])
            nc.scalar.copy(out=xpv[:, 7:15, 259:262], in_=xpv[:, 7:15, 257:254:-1])

            # ---- interior halo exchange on TensorE ----
            for i, (dst, src_n) in enumerate((((0, 3), (16, 19)), ((19, 22), (3, 6)))):
                m_shift = shmat[:, (2 * i) * P : (2 * i + 1) * P]
                m_self = shmat[:, (2 * i + 1) * P : (2 * i + 2) * P]
                ps = psp.tile([P, QF], F32, tag="ps", name=f"psh{i}")
                rflat = xpv[:, src_n[0] : src_n[1], :].rearrange("p r c -> p (r c)")
                sflat = xpv[:, dst[0] : dst[1], :].rearrange("p r c -> p (r c)")
                for n0, n1 in ((0, 512), (512, 786)):
                    nc.tensor.matmul(
                        ps[:, n0:n1], m_shift, rflat[:, n0:n1], start=True, stop=False
                    )
                    nc.tensor.matmul(
                        ps[:, n0:n1], m_self, sflat[:, n0:n1], start=False, stop=True
                    )
                nc.scalar.copy(
                    out=xpv[:, dst[0] : dst[1], :].rearrange("p r c -> p (r c)"),
                    in_=ps[:, 0:786],
                )

            # ---- s2 = sigma^2 then u = t (ACT); v/w interleave later ----
            s2 = iof("s2")
            s2v = s2[:].bitcast(F16)[:, 0:FMAP]
            u = maps.tile([P, FMAP], F16, tag="uvwr", name="u")
            v = maps.tile([P, FMAP], F16, tag="uvwr", name="v")
            w = maps.tile([P, FMAP], F16, tag="uvwr", name="w")
            rz = maps.tile([P, FMAP], F16, tag="uvwr", name="rz")
            nc.scalar.activation(s2v, sf32[:], AF.Square)
            nc.scalar.activation(u[:], s2v, AF.Exp, scale=-0.5)

            # ---- vertical pair sums V_a (DVE), full padded width ----
            vmaps = []
            for a in (1, 2, 3):
                t = vp.tile([P, BR * PADC], F16, tag="vt", name=f"v{a}")
                tv = t[:].rearrange("p (r c) -> p r c", r=BR)
                nc.vector.tensor_add(
                    tv, xpv[:, 3 - a : 3 - a + BR, :], xpv[:, 3 + a : 3 + a + BR, :]
                )
                vmaps.append(tv)
            V1, V2, V3 = vmaps
            X3 = xpv[:, 3 : 3 + BR, :]

            # ============ TensorE S-map builder ============
            def pe_map(name, srcs):
                vws = [s[:, :, 3 + d : 3 + d + W] for (s, d) in srcs]
                qts = []
                for q in range(NQ):
                    ps = psp.tile([P, QF], F32, tag="ps", name=f"ps_{name}{q}")
                    for h in range(2):
                        r0 = q * QR + 2 * h
                        for j, vw in enumerate(vws):
                            nc.tensor.matmul(
                                ps[:, h * 512 : (h + 1) * 512],
                                ident,
                                vw[:, r0 : r0 + 2, :],
                                start=(j == 0),
                                stop=(j == len(vws) - 1),
                            )
                    qt = ssbp.tile([P, QF], F16, tag="ssb", name=f"{name}q{q}")
                    nc.scalar.copy(out=qt[:], in_=ps[:])
                    qts.append(qt)
                return qts

            def cmap(src3, b, name):
                out = work.tile([P, FMAP], F16, tag="wk", name=name)
                nc.vector.tensor_add(
                    r3(out), src3[:, :, 3 - b : 3 - b + W], src3[:, :, 3 + b : 3 + b + W]
                )
                return out

            def tqmul(name, coef, qts):
                outs = []
                for q in range(NQ):
                    o = tqp.tile([P, QF], F16, tag="tq", bufs=12, name=f"{name}q{q}")
                    nc.vector.tensor_mul(o[:], coef[:, q * QF : (q + 1) * QF], qts[q][:])
                    outs.append(o)
                return outs

            # DVE: stencil C maps (no exp deps)
            C22 = cmap(V2, 2, "C22")
            C33 = cmap(V3, 3, "C33")
            C11 = cmap(V1, 1, "C11")

            # PE: S5 (needs V1,V2); its ACT copies run after the u-exp
            S5q = pe_map("s5", [(V1, -2), (V1, 2), (V2, -1), (V2, 1)])
            nc.scalar.activation(v[:], s2v, AF.Exp, scale=-2.0)

            # DVE: products in PE-consumption order
            t5q = tqmul("t5", u, S5q)
            t8 = tqp.tile([P, FMAP], F16, tag="tf", bufs=2, name="t8")
            nc.vector.tensor_mul(t8[:], v[:], C22[:])
            t2 = work.tile([P, FMAP], F16, tag="wk", name="t2")
            nc.vector.tensor_mul(t2[:], u[:], C11[:])

            S10q = pe_map("s10", [(V1, -3), (V1, 3), (V3, -1), (V3, 1)])
            nc.scalar.activation(w[:], s2v, AF.Exp, scale=-4.5)

            t10q = tqmul("t10", u, S10q)
            t18 = tqp.tile([P, FMAP], F16, tag="tf", bufs=2, name="t18")
            nc.vector.tensor_mul(t18[:], w[:], C33[:])

            S13q = pe_map("s13", [(V2, -3), (V2, 3), (V3, -2), (V3, 2)])
            t13q = tqmul("t13", v, S13q)

            # ============ combine: PE partial sums + DVE products ============
            outf = iof("outf")
            a1_views = [X3[:, :, 2 : 2 + W], X3[:, :, 4 : 4 + W], V1[:, :, 3 : 3 + W]]
            a2_views = [X3[:, :, 1 : 1 + W], X3[:, :, 5 : 5 + W], V2[:, :, 3 : 3 + W]]
            a3_views = [X3[:, :, 0 : 0 + W], X3[:, :, 6 : 6 + W], V3[:, :, 3 : 3 + W]]
            xc_view = X3[:, :, 3 : 3 + W]

            def pe_region(name, q, vws, flats):
                """flats: (tile, is_quarter_sized)"""
                ps = psp.tile([P, QF], F32, tag="ps", name=f"ps_{name}{q}")
                n = len(vws) + len(flats)
                for h in range(2):
                    r0 = q * QR + 2 * h
                    j = 0
                    for vw in vws:
                        nc.tensor.matmul(
                            ps[:, h * 512 : (h + 1) * 512],
                            ident,
                            vw[:, r0 : r0 + 2, :],
                            start=(j == 0),
                            stop=(j == n - 1),
                        )
                        j += 1
                    for fl, is_q in flats:
                        c0 = h * 512 if is_q else q * QF + h * 512
                        nc.tensor.matmul(
                            ps[:, h * 512 : (h + 1) * 512],
                            ident,
                            fl[:, c0 : c0 + 512],
                            start=(j == 0),
                            stop=(j == n - 1),
                        )
                        j += 1
                return ps

            def pmulq(name, coef, pss, dt=F16):
                outs = []
                for q, ps in enumerate(pss):
                    o = ppp.tile([P, QF], dt, tag="pp", bufs=12, name=f"{name}q{q}")
                    nc.vector.tensor_mul(o[:], coef[:, q * QF : (q + 1) * QF], ps[:])
                    outs.append(o)
                return outs

            psA2 = [
                pe_region("a2", q, a2_views, [(t5q[q], True), (t8, False)])
                for q in range(NQ)
            ]
            P2q = pmulq("P2", v, psA2)

            # ---- g = 1+2(u+v+w): halves on GpSimd and DVE ----
            gs = iof("gs")
            gsv = gs[:].bitcast(F16)[:, 0:FMAP]
            HF = FMAP // 2
            nc.gpsimd.tensor_add(gsv[:, 0:HF], u[:, 0:HF], v[:, 0:HF])
            nc.gpsimd.tensor_add(gsv[:, 0:HF], gsv[:, 0:HF], w[:, 0:HF])
            nc.vector.tensor_add(gsv[:, HF:FMAP], u[:, HF:FMAP], v[:, HF:FMAP])
            nc.vector.tensor_add(gsv[:, HF:FMAP], gsv[:, HF:FMAP], w[:, HF:FMAP])

            psA1 = [pe_region("a1", q, a1_views, [(t2, False)]) for q in range(NQ)]
            P1q = pmulq("P1", u, psA1)

            # ---- ln/rz (ACT, queued after the S copies) ----
            lng = iof("lng")
            nc.scalar.activation(lng[:], gsv, AF.Ln, bias=1.0, scale=2.0)
            nc.scalar.activation(rz[:], lng[:], AF.Exp, scale=-2.0)

            psA3 = [
                pe_region(
                    "a3", q, a3_views,
                    [(t10q[q], True), (t13q[q], True), (t18, False)],
                )
                for q in range(NQ)
            ]
            P3q = pmulq("P3", w, psA3)

            for q in range(NQ):
                qs = slice(q * QF, (q + 1) * QF)
                psO = pe_region(
                    "out", q, [xc_view],
                    [(P1q[q], True), (P2q[q], True), (P3q[q], True)],
                )
                nc.vector.tensor_mul(outf[:, qs], psO[:], rz[:, qs])
                nc.sync.dma_start(
                    out=ov[:, q * QR : (q + 1) * QR, :],
                    in_=r3(outf)[:, q * QR : (q + 1) * QR, :],
                )

    nc.compile()
    return nc


def make_in_maps(x, sigma):
    sh = np.zeros((P, 5 * P), np.float16)
    for m in range(P):
        if m % BANDS != 0:
            sh[m - 1, m] = 1.0          # top-shift
        else:
            sh[m, P + m] = 1.0          # top-self
        if m % BANDS != BANDS - 1:
            sh[m + 1, 2 * P + m] = 1.0  # bot-shift
        else:
            sh[m, 3 * P + m] = 1.0      # bot-self
        sh[m, 4 * P + m] = 1.0          # identity
    return [
        {
            "x": x[i * B_PER_CORE : (i + 1) * B_PER_CORE],
            "sigma": sigma[i * B_PER_CORE : (i + 1) * B_PER_CORE],
            "hshift": sh,
        }
        for i in range(N_CORES)
    ]


_NC_CACHE = None


def kernel(x: np.ndarray, sigma: np.ndarray) -> np.ndarray:
    global _NC_CACHE
    x = np.ascontiguousarray(np.asarray(x, dtype=np.float32))
    sigma = np.ascontiguousarray(np.asarray(sigma, dtype=np.float32))
    if _NC_CACHE is None:
        _NC_CACHE = build_nc()
    nc = _NC_CACHE
    in_maps = make_in_maps(x, sigma)
    res = run_bass_kernel_spmd(nc, in_maps, core_ids=list(range(N_CORES)))
    outs = [res.results[i]["out"] for i in range(N_CORES)]
    return np.concatenate(outs, axis=0).astype(np.float32)

